# revision 5
# baseline (speedup 1.0000x reference)
"""Trainium2 Bass kernel for the deterministic legality module.

Computes, for each board b, filter f and top-left placement (i,j):
    legal[b,f,i,j] = 1.0 iff every occupied cell of filter f, placed at
    (i,j), lands in-bounds on a free cell of board b (and f is non-empty).

Key structural facts exploited (all computed from the actual filter data
at kernel-build time, so the kernel stays correct for any filter set):

  * A placement (i,j) of filter f with max tap offsets (mdy, mdx) is
    structurally illegal (always 0) unless i <= 8-mdy and j <= 8-mdx.
    For random-ish 5x5 masks most filters have mdy = mdx = 4, so only
    ~1/3 of the 264*81 output columns are ever reachable.  The device
    computes ONLY those C columns; the host scatters into the zeroed
    full output.
  * Duplicate filter patterns share one device column.
  * legal = relu(corr + 1 - area) exactly (corr <= area, all integers),
    so the whole problem is ONE matmul over K = 82 (81 board cells + a
    threshold row of ones on the board side, thr = 1-area on the M side).
  * The 0/1 result is written to HBM as int8 (4x less write traffic than
    f32); the host widens while scattering.

M ([82, C] bf16, exact for these small integers) is built ON THE HOST
and DMA'd in -- there is no on-device build phase at all.

Sharding: pure data parallelism, batch 4096 -> 512 per core on 8 cores.
"""

import numpy as np
import ml_dtypes

N_CORES = 8
BATCH = 4096
BPC = BATCH // N_CORES  # 512 boards per core
NPOS = 81               # 9x9 board cells / placements
NF = 264                # filters
K = NPOS + 1            # contraction: 81 board cells + threshold row

COL_TILE = 512          # one PSUM bank of f32
COL_GROUP = 1024        # columns per drain / output DMA chunk
WARMUP_MM = 8           # dummy matmuls bridging input-DMA latency; also
                        # starts the PE activity monitor early


def _plan(filters):
    """Host-side column plan + M matrix from the actual filter data.

    Returns (M[K, c_pad] bf16, c_pad, f_sc, ij_sc, c_sc) where the
    scatter triplet satisfies full[:, f_sc, ij_sc] = raw[:, c_sc].
    """
    filt = np.asarray(filters, dtype=np.float32).reshape(NF, 5, 5)
    areas = filt.sum(axis=(1, 2))
    occ = filt > 0.5

    nonempty = np.where(areas > 0.5)[0]
    keys = (occ.reshape(NF, 25).astype(np.int64)
            * (1 << np.arange(25, dtype=np.int64))).sum(axis=1)
    _, first, inv = np.unique(keys[nonempty], return_index=True,
                              return_inverse=True)
    reps = nonempty[first]          # representative filter per pattern
    U = len(reps)

    mdy = np.array([occ[r].any(axis=1).nonzero()[0].max() for r in reps])
    mdx = np.array([occ[r].any(axis=0).nonzero()[0].max() for r in reps])

    # ij-major column list over unique patterns
    cols = []                       # (u, i, j)
    col_of = {}                     # (u, ij) -> c
    for i in range(9):
        for j in range(9):
            for u in range(U):
                if mdy[u] <= 8 - i and mdx[u] <= 8 - j:
                    col_of[(u, i * 9 + j)] = len(cols)
                    cols.append((u, i, j))
    C = len(cols)
    c_pad = -(-C // COL_TILE) * COL_TILE

    M = np.zeros((K, c_pad), dtype=np.float32)
    for c, (u, i, j) in enumerate(cols):
        r = reps[u]
        dys, dxs = np.nonzero(occ[r])
        M[(i + dys) * 9 + (j + dxs), c] = 1.0
        M[NPOS, c] = 1.0 - areas[r]

    f_sc, ij_sc, c_sc = [], [], []
    for fi, f in enumerate(nonempty):
        u = inv[fi]
        for ij in range(NPOS):
            c = col_of.get((u, ij))
            if c is not None:
                f_sc.append(f)
                ij_sc.append(ij)
                c_sc.append(c)
    return (M.astype(ml_dtypes.bfloat16), c_pad,
            np.asarray(f_sc), np.asarray(ij_sc), np.asarray(c_sc))


def _build_module(c_pad):
    import concourse.bass as bass
    import concourse.mybir as mybir
    import concourse.tile as tile
    from concourse.masks import make_identity

    f32 = mybir.dt.float32
    bf16 = mybir.dt.bfloat16
    i8 = mybir.dt.int8
    Relu = mybir.ActivationFunctionType.Relu

    nc = bass.Bass("TRN2", target_bir_lowering=False, debug=False,
                   num_devices=N_CORES)

    board_d = nc.dram_tensor("board", [BPC, NPOS], f32, kind="ExternalInput")
    m_d = nc.dram_tensor("mmat", [K, c_pad], bf16, kind="ExternalInput")
    out_d = nc.dram_tensor("out", [BPC, c_pad], i8, kind="ExternalOutput")

    # column-group boundaries (last group may be short)
    groups = []
    g0 = 0
    while g0 < c_pad:
        groups.append((g0, min(g0 + COL_GROUP, c_pad)))
        g0 += COL_GROUP

    with tile.TileContext(nc) as tc:
        with tc.tile_pool(name="const", bufs=1) as cpool:
            ident = cpool.tile([128, 128], f32)
            make_identity(nc, ident[:])

            M = cpool.tile([K, c_pad], bf16)
            boardT = cpool.tile([K, BPC], bf16)   # [81 cells + ones row, 512]

            # ---- input DMAs (all on SP) --------------------------------
            # board first (needed by the transposes at ~1us), then M
            # column-group-striped so group g unblocks as soon as its own
            # bytes land.  Early groups are split into partition stripes so
            # several DMA engines pull them concurrently.
            btile = None

            def emit_board_load(prep):
                nonlocal btile
                btile = prep.tile([128, 4 * K], f32, tag="bload")
                bt3 = btile[:].rearrange("p (c y) -> p c y", c=4)
                nc.gpsimd.memset(bt3[:, :, NPOS:K], 1.0)
                bview = board_d[:].rearrange("(c p) x -> p c x", p=128)
                nc.sync.dma_start(bt3[:, :, 0:NPOS], bview)

            def emit_m_loads():
                for gi, (g0, g1) in enumerate(groups):
                    nsplit = 4 if gi == 0 else (2 if gi <= 2 else 1)
                    step = -(-K // nsplit)
                    for p0 in range(0, K, step):
                        p1 = min(p0 + step, K)
                        nc.sync.dma_start(M[p0:p1, g0:g1],
                                          m_d[p0:p1, g0:g1])

            # ---- phase A: board load + transpose; PE warm-up -----------
            with (
                tc.tile_pool(name="prep", bufs=2) as prep,
                tc.tile_pool(name="psA", bufs=2, space="PSUM") as psA,
                tc.tile_pool(name="psW", bufs=1, space="PSUM") as psW,
            ):
                emit_board_load(prep)
                emit_m_loads()

                if WARMUP_MM:
                    wps = psW.tile([128, 128], f32, tag="warm")
                    for _ in range(WARMUP_MM):
                        nc.tensor.matmul(wps[:], ident[:], ident[:],
                                         start=True, stop=True)
                    wrd = prep.tile([32, 1], f32, tag="wrd")
                    nc.vector.tensor_scalar_add(wrd[:], wps[0:32, 0:1], 0.0)

                for kb in range(BPC // 128):
                    bps = psA.tile([K, 128], f32, tag="btp")
                    nc.tensor.transpose(bps[:], btile[:, kb * K:(kb + 1) * K],
                                        ident[:])
                    nc.vector.tensor_scalar_add(
                        boardT[:, kb * 128:(kb + 1) * 128], bps[:], 0.0)

            # ---- phase B: matmul + relu(int8) + store ------------------
            # group-outer / kb-inner: M group g+1 has ~3.4us to arrive
            # while group g computes.  Drains alternate DVE/ACT; output
            # DMAs ride the software DGE (gpsimd) so the SP sequencer
            # (628ns per trigger) stays off the critical path.
            with (
                tc.tile_pool(name="psM", bufs=4, space="PSUM") as psM,
                tc.tile_pool(name="ostage", bufs=6) as ostage,
            ):
                alt = 0
                nkb = BPC // 128
                last_g0 = groups[-1][0]
                for g0, g1 in groups:
                    gw = g1 - g0
                    for kb in range(nkb):
                        lhsT = boardT[:, kb * 128:(kb + 1) * 128]
                        pt = psM.tile([128, COL_GROUP], f32, tag="mm")
                        for off in range(0, gw, COL_TILE):
                            w = min(COL_TILE, gw - off)
                            nc.tensor.matmul(
                                pt[:, off:off + w], lhsT,
                                M[:, g0 + off:g0 + off + w],
                                start=True, stop=True)
                        ot = ostage.tile([128, COL_GROUP], i8, tag="ot")
                        if alt:
                            nc.scalar.activation(ot[:, :gw], pt[:, :gw], Relu)
                        else:
                            nc.vector.tensor_scalar_max(
                                ot[:, :gw], pt[:, :gw], 0.0)
                        alt ^= 1
                        rows = out_d[kb * 128:(kb + 1) * 128, g0:g1]
                        if g0 == last_g0:
                            # final wave: halve the chunks so the tail
                            # transfer after the last drain is short
                            nc.gpsimd.dma_start(rows[0:64, :], ot[0:64, :gw])
                            nc.gpsimd.dma_start(rows[64:128, :],
                                                ot[64:128, :gw])
                        else:
                            nc.gpsimd.dma_start(rows, ot[:, :gw])
    return nc


def _legalize_multiwait(nc):
    """Split multi-wait instructions for this walrus build.

    The TPB instruction encodings carry exactly one semaphore wait, and
    the walrus codegen here refuses instructions with more ("Too many
    sync wait commands").  Hoist all but one wait onto EventSemaphore
    carrier instructions placed immediately before, on the same engine —
    the sequencer blocks on each carrier first, which is semantically
    identical.
    """
    import concourse.mybir as mybir

    for func in nc.m.functions:
        for blk in func.blocks:
            out = []
            changed = False
            for inst in blk.instructions:
                si = inst.sync_info
                waits = list(si.on_wait) if si is not None and si.on_wait else []
                if len(waits) > 1:
                    for j, w in enumerate(waits[:-1]):
                        carrier = mybir.InstEventSemaphore(
                            name=f"{inst.name}-xw{j}",
                            engine=inst.engine,
                            ins=[], outs=[],
                            sync_info=mybir.SyncInfo(on_wait=[w],
                                                     on_update=[]),
                        )
                        nc.register_instruction(carrier)
                        out.append(carrier)
                    inst.sync_info = mybir.SyncInfo(
                        on_wait=[waits[-1]],
                        on_update=list(si.on_update) if si.on_update else [])
                    changed = True
                out.append(inst)
            if changed:
                blk.instructions = out


_CACHE = {}


def _get_module(c_pad):
    if c_pad not in _CACHE:
        nc = _build_module(c_pad)
        _legalize_multiwait(nc)
        _CACHE[c_pad] = nc
    return _CACHE[c_pad]


def run(board_free, filters, areas, trace=False, **spmd_kwargs):
    from concourse.bass_utils import run_bass_kernel_spmd

    M, c_pad, f_sc, ij_sc, c_sc = _plan(filters)

    board = np.ascontiguousarray(
        np.asarray(board_free, dtype=np.float32).reshape(N_CORES, BPC, NPOS))

    in_maps = [
        {"board": board[c], "mmat": M}
        for c in range(N_CORES)
    ]
    nc = _get_module(c_pad)
    res = run_bass_kernel_spmd(nc, in_maps, core_ids=list(range(N_CORES)),
                               trace=trace, **spmd_kwargs)
    raw = np.concatenate([r["out"] for r in res.results], axis=0)

    full = np.zeros((BATCH, NF, NPOS), dtype=np.float32)
    full[:, f_sc, ij_sc] = raw[:, c_sc]
    return full.reshape(BATCH, NF, 9, 9), res


def kernel(board_free, filters, areas):
    out, _ = run(board_free, filters, areas)
    return out


# revision 7
# speedup vs baseline: 1.3051x; 1.3051x over previous
"""Trainium2 Bass kernel for the deterministic legality module.

Computes, for each board b, filter f and top-left placement (i,j):
    legal[b,f,i,j] = 1.0 iff every occupied cell of filter f, placed at
    (i,j), lands in-bounds on a free cell of board b (and f is non-empty).

Structure exploited (all derived from the actual filter data at
kernel-build time, so the kernel stays correct for any filter set):

  * A placement (i,j) of filter f with max tap offsets (mdy, mdx) is
    structurally illegal unless i <= 8-mdy and j <= 8-mdx; only ~1/3 of
    the 264*81 output columns are reachable.  The device computes ONLY
    those C columns; the host scatters into the zeroed full output.
  * Duplicate filter patterns share one device column.
  * legal = relu(corr + thr) exactly, with the threshold rows folded
    into the contraction (board side carries ones rows).
  * The 0/1 result goes to HBM as int8 (4x less write traffic).

The matmul runs in fp8e4 DoubleRow mode (2 fp8 weights per PE cell,
half the streaming cycles of bf16).  Contraction K = 84 as two k-tiles
of 42: k = q*42+p; k 0..80 = board cells, k 81/82 = two threshold rows
(thr = 1-area split as ceil/floor halves so every value is e4m3-exact),
k 83 = zero pad.  M ([42, 2, C] fp8) is built ON THE HOST and DMA'd in;
there is no on-device build phase.

Sharding: pure data parallelism, batch 4096 -> 512 per core on 8 cores.
"""

import numpy as np
import ml_dtypes

N_CORES = 8
BATCH = 4096
BPC = BATCH // N_CORES  # 512 boards per core
NPOS = 81               # 9x9 board cells / placements
NF = 264                # filters
KT = 42                 # k-tile size (DoubleRow)
K = 2 * KT              # logical contraction: 81 cells + 2 thr + pad

COL_TILE = 512          # one PSUM bank of f32
COL_GROUP = 1024        # columns per drain / output DMA chunk
WARMUP_MM = 4           # dummy matmuls bridging input-DMA latency


def _plan(filters):
    """Host-side column plan + DoubleRow M matrix from the filter data.

    Returns (M[KT, 2*c_pad] fp8, c_pad, f_sc, ij_sc, c_sc) where the
    scatter triplet satisfies full[:, f_sc, ij_sc] = raw[:, c_sc].
    M[p, q*c_pad + c] holds logical row k = q*42+p of the legality
    matrix: taps for k<=80, thr_a at k=81, thr_b at k=82, zero at 83.
    """
    filt = np.asarray(filters, dtype=np.float32).reshape(NF, 5, 5)
    areas = filt.sum(axis=(1, 2))
    occ = filt > 0.5

    nonempty = np.where(areas > 0.5)[0]
    keys = (occ.reshape(NF, 25).astype(np.int64)
            * (1 << np.arange(25, dtype=np.int64))).sum(axis=1)
    _, first, inv = np.unique(keys[nonempty], return_index=True,
                              return_inverse=True)
    reps = nonempty[first]          # representative filter per pattern
    U = len(reps)

    mdy = np.array([occ[r].any(axis=1).nonzero()[0].max() for r in reps])
    mdx = np.array([occ[r].any(axis=0).nonzero()[0].max() for r in reps])

    cols = []                       # (u, i, j), ij-major
    col_of = {}                     # (u, ij) -> c
    for i in range(9):
        for j in range(9):
            for u in range(U):
                if mdy[u] <= 8 - i and mdx[u] <= 8 - j:
                    col_of[(u, i * 9 + j)] = len(cols)
                    cols.append((u, i, j))
    C = len(cols)
    c_pad = -(-C // COL_TILE) * COL_TILE

    M82 = np.zeros((K, c_pad), dtype=np.float32)
    for c, (u, i, j) in enumerate(cols):
        r = reps[u]
        dys, dxs = np.nonzero(occ[r])
        M82[(i + dys) * 9 + (j + dxs), c] = 1.0
        thr = 1.0 - areas[r]
        M82[NPOS, c] = np.ceil(thr / 2)       # thr_a, in [-12, 0]
        M82[NPOS + 1, c] = np.floor(thr / 2)  # thr_b, in [-12, 0]

    # interleave into DoubleRow k-tiles: M[p, q, c] = M82[q*42+p, c]
    M = np.ascontiguousarray(
        M82.reshape(2, KT, c_pad).transpose(1, 0, 2).reshape(KT, 2 * c_pad))

    f_sc, ij_sc, c_sc = [], [], []
    for fi, f in enumerate(nonempty):
        u = inv[fi]
        for ij in range(NPOS):
            c = col_of.get((u, ij))
            if c is not None:
                f_sc.append(f)
                ij_sc.append(ij)
                c_sc.append(c)
    return (M.astype(ml_dtypes.float8_e4m3fn), c_pad,
            np.asarray(f_sc), np.asarray(ij_sc), np.asarray(c_sc))


def _build_module(c_pad):
    import concourse.bass as bass
    import concourse.mybir as mybir
    import concourse.tile as tile
    from concourse.masks import make_identity

    f32 = mybir.dt.float32
    fp8 = mybir.dt.float8e4
    i8 = mybir.dt.int8
    Relu = mybir.ActivationFunctionType.Relu
    DR = mybir.MatmulPerfMode.DoubleRow

    nc = bass.Bass("TRN2", target_bir_lowering=False, debug=False,
                   num_devices=N_CORES)

    board_d = nc.dram_tensor("board", [BPC, NPOS], f32, kind="ExternalInput")
    m_d = nc.dram_tensor("mmat", [KT, 2 * c_pad], fp8, kind="ExternalInput")
    out_d = nc.dram_tensor("out", [BPC, c_pad], i8, kind="ExternalOutput")

    groups = []
    g0 = 0
    while g0 < c_pad:
        groups.append((g0, min(g0 + COL_GROUP, c_pad)))
        g0 += COL_GROUP
    nkb = BPC // 128

    with tile.TileContext(nc) as tc:
        with tc.tile_pool(name="const", bufs=1) as cpool:
            ident = cpool.tile([128, 128], f32)
            make_identity(nc, ident[:])

            M = cpool.tile([KT, 2 * c_pad], fp8)
            M3 = M[:].rearrange("p (q n) -> p q n", q=2)
            m3 = m_d[:].rearrange("p (q n) -> p q n", q=2)
            boardT = cpool.tile([KT, 2 * BPC], fp8)
            bT3 = boardT[:].rearrange("p (q n) -> p q n", q=2)

            # ---- input DMA chunking ------------------------------------
            # M group g must land before the main loop reaches it
            # (~2.2us per group).  Early groups are split into partition
            # halves / quarters; triggers are spread over DVE and ACT
            # which are otherwise idle until the first drains (~4us).
            # SP carries the board load, the late M singles, and all
            # late output chunks.
            def m_load(eng, p0, p1, g0, g1):
                eng.dma_start(M3[p0:p1, :, g0:g1], m3[p0:p1, :, g0:g1])

            def emit_m_loads():
                # HWDGE triggers exist on SP and ACT only; DVE has none.
                h = KT // 2
                for gi, g in enumerate(groups[:3]):
                    m_load(nc.sync, 0, h, *g)
                    m_load(nc.scalar, h, KT, *g)
                for g in groups[3:]:
                    m_load(nc.sync, 0, KT, *g)

            # ---- phase A: board load + transpose; PE warm-up -----------
            with (
                tc.tile_pool(name="prep", bufs=2) as prep,
                tc.tile_pool(name="psA", bufs=2, space="PSUM") as psA,
                tc.tile_pool(name="psW", bufs=1, space="PSUM") as psW,
            ):
                # board (512,81) f32 -> btile rows, 84 cols per board:
                # [cells 0..80, 1.0, 1.0, 0.0]; two 42-col transposes per
                # 128-board block produce the two k-tiles at partition 0.
                btile = prep.tile([128, 4 * K], f32, tag="bload")
                bt3 = btile[:].rearrange("p (c y) -> p c y", c=4)
                nc.gpsimd.memset(bt3[:, :, NPOS:NPOS + 2], 1.0)
                nc.gpsimd.memset(bt3[:, :, NPOS + 2:K], 0.0)
                bview = board_d[:].rearrange("(c p) x -> p c x", p=128)
                nc.sync.dma_start(bt3[:, :, 0:NPOS], bview)

                emit_m_loads()

                if WARMUP_MM:
                    wps = psW.tile([128, 128], f32, tag="warm")
                    for _ in range(WARMUP_MM):
                        nc.tensor.matmul(wps[:], ident[:], ident[:],
                                         start=True, stop=True)
                    wrd = prep.tile([32, 1], f32, tag="wrd")
                    nc.vector.tensor_scalar_add(wrd[:], wps[0:32, 0:1], 0.0)

                for kb in range(nkb):
                    bps = psA.tile([KT, 256], f32, tag="btp")
                    for q in range(2):
                        nc.tensor.transpose(
                            bps[:, q * 128:(q + 1) * 128],
                            btile[:, kb * K + q * KT:kb * K + (q + 1) * KT],
                            ident[:])
                    src = bps[:].rearrange("p (q n) -> p q n", q=2)
                    nc.scalar.copy(bT3[:, :, kb * 128:(kb + 1) * 128], src)

            # ---- phase B: DoubleRow matmul + relu(int8) + store --------
            with (
                tc.tile_pool(name="psM", bufs=4, space="PSUM") as psM,
                tc.tile_pool(name="ostage", bufs=6) as ostage,
            ):
                alt = 0
                last_g0 = groups[-1][0]
                for g0, g1 in groups:
                    gw = g1 - g0
                    for kb in range(nkb):
                        lhsT = bT3[:, :, kb * 128:(kb + 1) * 128]
                        pt = psM.tile([128, COL_GROUP], f32, tag="mm")
                        for off in range(0, gw, COL_TILE):
                            w = min(COL_TILE, gw - off)
                            nc.tensor.matmul(
                                pt[:, off:off + w], lhsT,
                                M3[:, :, g0 + off:g0 + off + w],
                                start=True, stop=True, perf_mode=DR)
                        ot = ostage.tile([128, COL_GROUP], i8, tag="ot")
                        if alt:
                            nc.scalar.activation(ot[:, :gw], pt[:, :gw], Relu)
                        else:
                            nc.vector.tensor_scalar_max(
                                ot[:, :gw], pt[:, :gw], 0.0)
                        alt ^= 1
                        rows = out_d[kb * 128:(kb + 1) * 128, g0:g1]
                        if g0 == last_g0:
                            # final wave: small chunks for a short tail
                            nc.sync.dma_start(rows[0:64, :], ot[0:64, :gw])
                            nc.sync.dma_start(rows[64:128, :],
                                              ot[64:128, :gw])
                        elif g0 < 3 * COL_GROUP:
                            # early chunks ride the software DGE; its
                            # ~1us/desc-gen latency is hidden behind the
                            # main loop and it keeps SP free
                            nc.gpsimd.dma_start(rows, ot[:, :gw])
                        else:
                            nc.sync.dma_start(rows, ot[:, :gw])
    return nc


def _legalize_multiwait(nc):
    """Split multi-wait instructions for this walrus build.

    The TPB instruction encodings carry exactly one semaphore wait, and
    the walrus codegen here refuses instructions with more ("Too many
    sync wait commands").  Hoist all but one wait onto EventSemaphore
    carrier instructions placed immediately before, on the same engine —
    the sequencer blocks on each carrier first, which is semantically
    identical.
    """
    import concourse.mybir as mybir

    for func in nc.m.functions:
        for blk in func.blocks:
            out = []
            changed = False
            for inst in blk.instructions:
                si = inst.sync_info
                waits = list(si.on_wait) if si is not None and si.on_wait else []
                if len(waits) > 1:
                    for j, w in enumerate(waits[:-1]):
                        carrier = mybir.InstEventSemaphore(
                            name=f"{inst.name}-xw{j}",
                            engine=inst.engine,
                            ins=[], outs=[],
                            sync_info=mybir.SyncInfo(on_wait=[w],
                                                     on_update=[]),
                        )
                        nc.register_instruction(carrier)
                        out.append(carrier)
                    inst.sync_info = mybir.SyncInfo(
                        on_wait=[waits[-1]],
                        on_update=list(si.on_update) if si.on_update else [])
                    changed = True
                out.append(inst)
            if changed:
                blk.instructions = out


_CACHE = {}


def _get_module(c_pad):
    if c_pad not in _CACHE:
        nc = _build_module(c_pad)
        _legalize_multiwait(nc)
        _CACHE[c_pad] = nc
    return _CACHE[c_pad]


def run(board_free, filters, areas, trace=False, **spmd_kwargs):
    from concourse.bass_utils import run_bass_kernel_spmd

    M, c_pad, f_sc, ij_sc, c_sc = _plan(filters)

    board = np.ascontiguousarray(
        np.asarray(board_free, dtype=np.float32).reshape(N_CORES, BPC, NPOS))

    in_maps = [
        {"board": board[c], "mmat": M}
        for c in range(N_CORES)
    ]
    nc = _get_module(c_pad)
    res = run_bass_kernel_spmd(nc, in_maps, core_ids=list(range(N_CORES)),
                               trace=trace, **spmd_kwargs)
    raw = np.concatenate([r["out"] for r in res.results], axis=0)

    full = np.zeros((BATCH, NF, NPOS), dtype=np.float32)
    full[:, f_sc, ij_sc] = raw[:, c_sc]
    return full.reshape(BATCH, NF, 9, 9), res


def kernel(board_free, filters, areas):
    out, _ = run(board_free, filters, areas)
    return out
